# revision 1
# baseline (speedup 1.0000x reference)
"""Bass/Tile kernel builder for nn_MultiMetricPredictor.

Per-core: 128 samples x 120 tokens. Encoder (2 layers) + attention-pool +
ctx + 90-step GRU decode. bf16 matmuls, fp32 psum accumulate.

Layouts:
  h (residual stream): token-major [120 tok-part, 128 samples, 128 feat] bf16
  attention q/k feature-major [128, 120] per sample; v token-major; scores
  Tk-major [120, 4*120]; AV col-tiled; softmax denominator via all-ones MM,
  divide folded into the o psum->sbuf copy.
  GRU: gates feature-major [128 gate-feat, 128 samples]; state hd bf16
  feature-major; m2 head token-major tail; pred transposed back via PE.
ACT tables: encoder uses exp/ln only (natural_log_exp set); GRU uses
  sigmoid/tanh/erf (sigmoid_and_others set).
"""
import math
import numpy as np
import ml_dtypes

import concourse.mybir as mybir
from concourse.masks import make_identity

F32 = mybir.dt.float32
BF16 = mybir.dt.bfloat16
AF = mybir.ActivationFunctionType
OP = mybir.AluOpType

B, T, F = 1024, 120, 32
D, H, L, HD = 128, 4, 2, 32
SD, RD, M, HOR = 16, 8, 5, 90
NCORES = 8
BC = B // NCORES          # 128 samples/core
NTOK = BC * T             # 15360

LN2C = float(np.log(2.0))
ISQ2 = float(1.0 / np.sqrt(2.0))


def _bf(x):
    return np.ascontiguousarray(np.asarray(x, np.float32).astype(ml_dtypes.bfloat16))


def _f32(x):
    return np.ascontiguousarray(np.asarray(x, np.float32))


def _sinusoidal():
    pos = np.arange(T, dtype=np.float64)[:, None]
    div = np.exp(np.arange(0, D, 2, dtype=np.float64) * (-math.log(10000.0) / D))
    pe = np.zeros((T, D))
    pe[:, 0::2] = np.sin(pos * div)
    pe[:, 1::2] = np.cos(pos * div)
    return pe


def host_prep(inputs):
    """Returns (shared weight/const dict, list of per-core input dicts)."""
    inputs = {k: np.asarray(v) for k, v in inputs.items()}
    w = {}
    inw = _f32(inputs["in_w"])          # [128, 48]
    w["inwT"] = _bf(inw.T)              # [48, 128]
    assert not np.any(inputs["in_b"]), "nonzero in_b: fold not implemented"

    w["pe_t"] = _bf(_sinusoidal())      # [120, 128]

    for l in range(L):
        ln1w = _f32(inputs["enc_ln1_w"][l]); ln1b = _f32(inputs["enc_ln1_b"][l])
        ln2w_ = _f32(inputs["enc_ln2_w"][l]); ln2b = _f32(inputs["enc_ln2_b"][l])
        assert not (np.any(ln1b) or np.any(ln2b) or np.any(inputs["enc_qkv_b"][l])
                    or np.any(inputs["enc_out_b"][l]) or np.any(inputs["enc_f1_b"][l])
                    or np.any(inputs["enc_f2_b"][l])), "nonzero encoder bias"
        qkv_eff = _f32(inputs["enc_qkv_w"][l]) * ln1w[None, :]
        w[f"wqT{l}"] = _bf(qkv_eff[0:128].T / math.sqrt(HD))
        w[f"wkT{l}"] = _bf(qkv_eff[128:256].T)
        w[f"wvT{l}"] = _bf(qkv_eff[256:384].T)
        w[f"woT{l}"] = _bf(_f32(inputs["enc_out_w"][l]).T)
        f1 = _f32(inputs["enc_f1_w"][l]) * ln2w_[None, :]   # [512, 128]
        w[f"w1T{l}"] = _bf(f1.T)                 # [128, 512]; chunk j = cols 128j..
        f2 = _f32(inputs["enc_f2_w"][l])         # [128, 512]
        w2t = np.concatenate([f2[:, 128 * j:128 * (j + 1)].T for j in range(4)], axis=1)
        w[f"w2T{l}"] = _bf(w2t)                  # [128, 512]

    # pool_b shifts all logits equally -> softmax invariant; skip it.
    w["pwbc"] = _bf(np.broadcast_to(_f32(inputs["pool_w"])[0][None, :], (T, D)))

    cw = _f32(inputs["ctx_w"])                   # [128, 152]
    w["ctxTp"] = _bf(cw[:, 0:128].T)
    w["ctxTs"] = _bf(cw[:, 128:144].T)           # [16, 128]
    w["ctxTr"] = _bf(cw[:, 144:152].T)           # [8, 128]
    w["ctxb"] = _f32(inputs["ctx_b"]).reshape(128, 1)

    wih = _f32(inputs["gru_wih"])                # [384, 133]
    whh = _f32(inputs["gru_whh"])                # [384, 128]
    bih = _f32(inputs["gru_bih"]); bhh = _f32(inputs["gru_bhh"])
    flags = {}
    for gi, g in enumerate("rzn"):
        blk = slice(128 * gi, 128 * (gi + 1))
        w[f"whhT_{g}"] = _bf(whh[blk].T)         # [128, 128]
        w[f"wih5_{g}"] = _bf(wih[blk, 0:5].T)    # [5, 128]
        w[f"wihcT_{g}"] = _bf(wih[blk, 5:133].T)  # [128, 128]
        bb = bih[blk] + (bhh[blk] if g in "rz" else 0.0)
        w[f"gicb_{g}"] = _f32(bb.reshape(1, 128))
        flags[f"gicb_{g}"] = bool(np.any(bb))
    w["bhh_n"] = _f32(bhh[256:384].reshape(1, 128))
    flags["bhh_n"] = bool(np.any(w["bhh_n"]))

    mu1 = _f32(inputs["mu_w1"]); vo1 = _f32(inputs["vol_w1"])   # [64, 128]
    w["wmv1T"] = _bf(np.concatenate([mu1, vo1], 0).T)           # [128, 128]
    w["mvb1"] = _f32(np.concatenate([inputs["mu_b1"], inputs["vol_b1"]]).reshape(1, 128))
    flags["mvb1"] = bool(np.any(w["mvb1"]))
    mu2 = _f32(inputs["mu_w2"]); vo2 = _f32(inputs["vol_w2"])   # [5, 64]
    wmv2 = np.zeros((128, 10), np.float32)
    wmv2[0:64, 0:5] = 0.5 * mu2.T
    wmv2[64:128, 5:10] = 0.5 * vo2.T
    w["wmv2"] = _bf(wmv2)
    w["mvb2"] = _bf(np.concatenate([inputs["mu_b2"], inputs["vol_b2"]]).reshape(1, 10))
    flags["mvb2"] = bool(np.any(_f32(w["mvb2"])))
    w["_flags"] = flags

    x = _f32(inputs["x"])
    se_all = _f32(inputs["sym_emb"][inputs["sym_id"]])   # [1024, 16]
    re_all = _f32(inputs["reg_emb"][inputs["regime_id"]])
    rv = np.std(x[:, :, 0].astype(np.float64), axis=1, ddof=1).astype(np.float32)

    cores = []
    for c in range(NCORES):
        sl = slice(c * BC, (c + 1) * BC)
        xa = np.concatenate(
            [x[sl], np.broadcast_to(se_all[sl][:, None, :], (BC, T, SD))], axis=-1)
        cores.append({
            "xa": _bf(xa.transpose(2, 0, 1).reshape(48, NTOK)),
            "se": _bf(se_all[sl].T),
            "re": _bf(re_all[sl].T),
            "rva": _f32(np.stack([(1 + rv[sl]) * LN2C,
                                  (1 + rv[sl]) * 0.5,
                                  (1 + rv[sl]) * 0.125], axis=1)),
        })
    return w, cores


def build(nc, w, dbg=(), reps=1):
    """dbg: list of (name, shape, 'f32'|'bf16') intermediates to expose."""
    import concourse.tile as tile

    dram = {}

    def din(name, arr):
        dt = BF16 if arr.dtype == ml_dtypes.bfloat16 else F32
        t = nc.dram_tensor(name, list(arr.shape), dt, kind="ExternalInput")
        dram[name] = t
        return t

    wd = {k: din(k, v) for k, v in w.items() if isinstance(v, np.ndarray)}
    import numpy as _np
    wd["xa"] = din("xa", _np.zeros((48, NTOK), ml_dtypes.bfloat16))
    wd["se"] = din("se", _np.zeros((16, BC), ml_dtypes.bfloat16))
    wd["re"] = din("re", _np.zeros((8, BC), ml_dtypes.bfloat16))
    wd["rva"] = din("rva", _np.zeros((BC, 3), _np.float32))
    d_out = nc.dram_tensor("preds", [BC, HOR * M], F32, kind="ExternalOutput")
    dram["preds"] = d_out
    d_dbg = {}
    for name, shape, kind in dbg:
        d_dbg[name] = nc.dram_tensor(
            "dbg_" + name, list(shape), BF16 if kind == "bf16" else F32,
            kind="ExternalOutput")
        dram["dbg_" + name] = d_dbg[name]

    with tile.TileContext(nc) as tc:
        if reps == 1:
            _body(nc, tc, w, wd, d_out, d_dbg)
        else:
            with tc.For_i(0, reps, 1):
                _body(nc, tc, w, wd, d_out, d_dbg)
    return dram


def _body(nc, tc, w, wd, d_out, d_dbg):
    import os
    import contextlib
    STAGE = int(os.environ.get("KSTAGE", "6"))
    flags = w["_flags"]

    def sbuf(name, shape, dtype):
        return nc.alloc_sbuf_tensor(name, list(shape), dtype).ap()

    xa_sb = sbuf("xa_sb", (48, NTOK), BF16)
    h_a = sbuf("h_a", (T, BC, D), BF16)
    h_m = sbuf("h_m", (T, BC, D), BF16)
    h_b = sbuf("h_b", (T, BC, D), BF16)
    mv_all = sbuf("mv_all", (T, BC, 2), F32)
    rstd_all = sbuf("rstd_all", (T, BC), F32)
    plog = sbuf("plog", (T, BC), F32)
    pexp = sbuf("pexp", (T, BC), BF16)
    preds_all = sbuf("preds_all", (BC, HOR * M), F32)
    hd_bf = sbuf("hd_bf", (D, BC), BF16)
    pred_bf = sbuf("pred_bf", (M, BC), BF16)
    gic = {g: sbuf(f"gic_{g}", (D, BC), BF16) for g in "rzn"}
    ctx_bf = sbuf("ctx_bf", (D, BC), BF16)

    MM = nc.tensor.matmul

    def dump(name, ap):
        if name in d_dbg:
            nc.sync.dma_start(d_dbg[name][:], ap)

    with tc.tile_pool(name="singles", bufs=1) as singles:
        i120 = singles.tile([T, T], BF16)
        make_identity(nc, i120)
        i128b = singles.tile([D, D], BF16)
        make_identity(nc, i128b)
        i128f = singles.tile([D, D], F32)
        make_identity(nc, i128f)
        ones_t1 = singles.tile([T, 1], BF16)
        nc.vector.memset(ones_t1, 1.0)
        ones_t32 = singles.tile([T, 32], BF16)
        nc.vector.memset(ones_t32, 1.0)
        ones_1b_f = singles.tile([1, BC], F32)
        nc.vector.memset(ones_1b_f, 1.0)
        ones_1b_bf = singles.tile([1, BC], BF16)
        nc.vector.memset(ones_1b_bf, 1.0)
        eps_t = singles.tile([T, 1], F32)
        nc.vector.memset(eps_t, 1e-5)

        nc.sync.dma_start(xa_sb, wd["xa"][:])
        ws = {}
        for k, t in wd.items():
            if k == "xa":
                continue
            shape = list(t.shape)
            dt = t.dtype
            tl = singles.tile(shape, dt, tag="w_" + k)
            nc.sync.dma_start(tl, t[:])
            ws[k] = tl

        copy_engines = [nc.vector, nc.scalar]

        def copy(dst, src, i=0):
            eng = copy_engines[i % len(copy_engines)]
            if eng is nc.scalar:
                nc.scalar.activation(dst, src, AF.Identity)
            else:
                eng.tensor_copy(dst, src)

        # ---------------- input projection ----------------
        with tc.tile_pool(name="projp", bufs=4, space="PSUM") as projp:
            for s in range(BC):
                ps = projp.tile([T, D], F32, tag="proj")
                MM(ps, xa_sb[:, s * T:(s + 1) * T], ws["inwT"], start=True, stop=False)
                MM(ps, i120, ws["pe_t"], start=False, stop=True)
                copy(h_a[:, s, :], ps, s)
        dump("h1", h_a)
        if STAGE < 2:
            nc.sync.dma_start(d_out[:], preds_all)
            return

        def ln_pass(h_in, tp):
            for s in range(BC):
                st = tp.tile([T, 6], F32, tag="bnst")
                nc.vector.bn_stats(st, h_in[:, s, :])
                nc.vector.bn_aggr(mv_all[:, s, :], st)
            lnv = tp.tile([T, BC], F32, tag="lnv")
            nc.scalar.activation(lnv, mv_all[:, :, 1], AF.Ln, bias=eps_t, scale=1.0)
            nc.scalar.activation(rstd_all, lnv, AF.Exp, scale=-0.5)

        ASTAGE = int(os.environ.get("KASTAGE", "9"))

        def attn_sublayer(l, h_in, h_mid, tsb, tsb2):
            wq, wk, wv, wo = ws[f"wqT{l}"], ws[f"wkT{l}"], ws[f"wvT{l}"], ws[f"woT{l}"]
            ln_pass(h_in, tsb)
            with tc.tile_pool(name="ap1", bufs=1, space="PSUM") as ap1, \
                 tc.tile_pool(name="ap2", bufs=1, space="PSUM") as ap2:
                for s in range(BC):
                    y1n = tsb.tile([T, D], BF16, tag="y1n")
                    nc.vector.tensor_scalar(y1n, h_in[:, s, :], mv_all[:, s, 0:1],
                                            rstd_all[:, s:s + 1],
                                            op0=OP.subtract, op1=OP.mult)
                    if ASTAGE <= 1:
                        nc.vector.tensor_copy(h_mid[:, s, :], y1n)
                        continue
                    trp = ap1.tile([D, T], BF16, tag="trp")
                    nc.tensor.transpose(trp, y1n, i120)
                    y1f = tsb.tile([D, T], BF16, tag="y1f")
                    copy(y1f, trp, s)
                    aw = ap1.tile([D, 368], F32, tag="aw")
                    MM(aw[:, 0:T], wq, y1f, start=True, stop=True)
                    MM(aw[:, T:2 * T], wk, y1f, start=True, stop=True)
                    MM(aw[0:T, 240:240 + D], y1f, wv, start=True, stop=True)
                    qk = tsb.tile([D, 2 * T], BF16, tag="qksb")
                    copy(qk, aw[:, 0:240], s + 1)
                    v = tsb.tile([T, D], BF16, tag="vsb")
                    copy(v, aw[0:T, 240:240 + D], s + 2)
                    if ASTAGE <= 2:
                        nc.vector.tensor_copy(h_mid[:, s, :], v)
                        continue
                    sT = ap2.tile([T, H, 512], F32, tag="sT")
                    NHEAD = int(os.environ.get("KNHEAD", str(H)))
                    for hh in range(NHEAD):
                        MM(sT[:, hh, 0:T],
                           qk[32 * hh:32 * (hh + 1), T:2 * T],
                           qk[32 * hh:32 * (hh + 1), 0:T],
                           start=True, stop=True, tile_position=(32 * hh, 0))
                    for hh in range(NHEAD, H):
                        nc.vector.memset(sT[:, hh, 0:T], 0.0)
                    e = tsb2.tile([T, H * T], BF16, tag="esb")
                    nc.scalar.activation(e.rearrange("t (h q) -> t h q", h=H),
                                         sT[:, :, 0:T], AF.Exp)
                    if ASTAGE <= 3:
                        nc.vector.tensor_copy(h_mid[:, s, 0:T], e[:, 0:T])
                        nc.vector.memset(h_mid[:, s, T:D], 0.0)
                        continue
                    od = ap1.tile([D, 2 * T], F32, tag="od")
                    for hh in range(H):
                        MM(od[32 * hh:32 * (hh + 1), 0:T],
                           v[:, 32 * hh:32 * (hh + 1)], e[:, hh * T:(hh + 1) * T],
                           start=True, stop=True, tile_position=(0, 32 * hh))
                        MM(od[32 * hh:32 * (hh + 1), T:2 * T],
                           ones_t32, e[:, hh * T:(hh + 1) * T],
                           start=True, stop=True, tile_position=(0, 32 * hh))
                    rd = tsb2.tile([D, T], F32, tag="rd")
                    nc.vector.reciprocal(rd, od[:, T:2 * T])
                    o_n = tsb2.tile([D, T], BF16, tag="on")
                    nc.vector.tensor_tensor(o_n, od[:, 0:T], rd, OP.mult)
                    h2 = ap2.tile([T, D], F32, tag="h2")
                    MM(h2, o_n, wo, start=True, stop=False)
                    MM(h2, i120, h_in[:, s, :], start=False, stop=True)
                    copy(h_mid[:, s, :], h2, s)

        def ffn_sublayer(l, h_mid, h_out, tsb, tsb2):
            w1, w2 = ws[f"w1T{l}"], ws[f"w2T{l}"]
            ln_pass(h_mid, tsb)
            with tc.tile_pool(name="fp1", bufs=2, space="PSUM") as fp1, \
                 tc.tile_pool(name="fp2", bufs=2, space="PSUM") as fp2:
                for s in range(BC):
                    y2n = tsb.tile([T, D], BF16, tag="y1n")
                    nc.vector.tensor_scalar(y2n, h_mid[:, s, :], mv_all[:, s, 0:1],
                                            rstd_all[:, s:s + 1],
                                            op0=OP.subtract, op1=OP.mult)
                    ytr = fp1.tile([D, T], BF16, tag="ytr")
                    nc.tensor.transpose(ytr, y2n, i120)
                    y2f = tsb.tile([D, T], BF16, tag="y1f")
                    copy(y2f, ytr, s)
                    rps = fp2.tile([D, 4 * T], F32, tag="rps")
                    for j in range(4):
                        MM(rps[:, j * T:(j + 1) * T], w1[:, 128 * j:128 * (j + 1)],
                           y2f, start=True, stop=True)
                    rr = tsb2.tile([D, 4 * T], BF16, tag="rr")
                    nc.vector.tensor_scalar_max(rr, rps, 0.0)
                    h3 = fp1.tile([T, D], F32, tag="h3")
                    for j in range(4):
                        MM(h3, rr[:, j * T:(j + 1) * T], w2[:, 128 * j:128 * (j + 1)],
                           start=(j == 0), stop=False)
                    MM(h3, i120, h_mid[:, s, :], start=False, stop=True)
                    copy(h_out[:, s, :], h3, s)

        with tc.tile_pool(name="tsb", bufs=4) as tsb, \
             tc.tile_pool(name="tsb2", bufs=4) as tsb2:
            attn_sublayer(0, h_a, h_m, tsb, tsb2)
            dump("h2a", h_m)
            if STAGE >= 3:
                ffn_sublayer(0, h_m, h_b, tsb, tsb2)
                dump("h2", h_b)
            if STAGE >= 4:
                attn_sublayer(1, h_b, h_m, tsb, tsb2)
                ffn_sublayer(1, h_m, h_a, tsb, tsb2)
            h_fin = h_a
            if STAGE >= 4:
                dump("h3", h_fin)
            if STAGE < 5:
                nc.sync.dma_start(d_out[:], preds_all)
                return

            # ---------------- pooling + ctx ----------------
            PSTAGE = int(os.environ.get("KPSTAGE", "9"))
            with tc.tile_pool(name="pl1", bufs=1, space="PSUM") as pl1:
                for s in range(BC):
                    scr = tsb.tile([T, D], F32, tag="pscr")
                    nc.vector.tensor_tensor(scr, h_fin[:, s, :], ws["pwbc"], OP.mult)
                    nc.vector.tensor_reduce(plog[:, s:s + 1], scr,
                                            mybir.AxisListType.X, OP.add)
                nc.scalar.activation(pexp, plog, AF.Exp)
                if PSTAGE < 2:
                    nc.sync.dma_start(
                        d_out.rearrange("b q -> (b q)")[0:T * BC]
                             .rearrange("(t b) -> t b", t=T), plog)
                    return
                dsum = pl1.tile([1, BC], F32, tag="dsum")
                MM(dsum, ones_t1, pexp, start=True, stop=True)
                prd = tsb.tile([1, BC], F32, tag="prd")
                nc.vector.reciprocal(prd, dsum)
                rdbc = pl1.tile([D, BC], F32, tag="rdbc")
                MM(rdbc, ones_1b_f, prd, start=True, stop=True)
                if PSTAGE < 3:
                    nc.sync.dma_start(
                        d_out.rearrange("b q -> (b q)")[0:BC].rearrange("(o b) -> o b", o=1), prd)
                    return
                pooled = pl1.tile([D, BC], F32, tag="pooled")
                for s in range(BC):
                    MM(pooled[:, s:s + 1], h_fin[:, s, :], pexp[:, s:s + 1],
                       start=True, stop=True)
                if PSTAGE < 4:
                    t_ = tsb.tile([D, BC], F32, tag="dbgp")
                    nc.vector.tensor_copy(t_, pooled)
                    nc.sync.dma_start(
                        d_out.rearrange("b q -> (b q)")[0:D * BC].rearrange("(d b) -> d b", d=D), t_)
                    return
                rdbc_sb = tsb.tile([D, BC], F32, tag="rdbcsb")
                nc.vector.tensor_copy(rdbc_sb, rdbc)
                pooled_n = tsb.tile([D, BC], BF16, tag="pooledn")
                nc.vector.tensor_tensor(pooled_n, pooled, rdbc_sb, OP.mult)
                ctxps = pl1.tile([D, BC], F32, tag="ctxps")
                MM(ctxps, ws["ctxTp"], pooled_n, start=True, stop=False)
                MM(ctxps, ws["ctxTs"], ws["se"], start=False, stop=False)
                MM(ctxps, ws["ctxTr"], ws["re"], start=False, stop=True)
                nc.scalar.activation(ctx_bf, ctxps, AF.Identity, bias=ws["ctxb"])
                dump("ctx", ctx_bf)
                for gi_, g in enumerate("rzn"):
                    gps = pl1.tile([D, BC], F32, tag="gicps")
                    MM(gps, ws[f"wihcT_{g}"], ctx_bf,
                       start=True, stop=not flags[f"gicb_{g}"])
                    if flags[f"gicb_{g}"]:
                        MM(gps, ws[f"gicb_{g}"], ones_1b_f, start=False, stop=True)
                    copy(gic[g], gps, gi_)

        if STAGE < 6:
            nc.sync.dma_start(d_out[:], preds_all)
            return
        # ---------------- GRU ----------------
        a0 = ws["rva"][:, 0:1]
        a1 = ws["rva"][:, 1:2]
        a2 = ws["rva"][:, 2:3]
        nc.vector.tensor_copy(hd_bf, ctx_bf)
        nc.vector.memset(pred_bf, 0.0)
        with tc.tile_pool(name="gq", bufs=2, space="PSUM") as gq, \
             tc.tile_pool(name="gq1", bufs=1, space="PSUM") as gq1, \
             tc.tile_pool(name="gp", bufs=2) as gp:
            for t in range(HOR):
                rz_ps = gq.tile([D, 2 * BC], F32, tag="rzps")
                for gi_, g in enumerate("rz"):
                    o = rz_ps[:, gi_ * BC:(gi_ + 1) * BC]
                    MM(o, ws[f"whhT_{g}"], hd_bf, start=True, stop=False)
                    MM(o, ws[f"wih5_{g}"], pred_bf, start=False, stop=False)
                    MM(o, i128b, gic[g], start=False, stop=True)
                n_ps = gq.tile([D, 2 * BC], F32, tag="nps")
                MM(n_ps[:, 0:BC], ws["whhT_n"], hd_bf,
                   start=True, stop=not flags["bhh_n"])
                if flags["bhh_n"]:
                    MM(n_ps[:, 0:BC], ws["bhh_n"], ones_1b_f, start=False, stop=True)
                MM(n_ps[:, BC:2 * BC], ws["wih5_n"], pred_bf, start=True, stop=False)
                MM(n_ps[:, BC:2 * BC], i128b, gic["n"], start=False, stop=True)
                rz_bf = gp.tile([D, 2 * BC], BF16, tag="rzbf")
                nc.scalar.activation(rz_bf, rz_ps, AF.Sigmoid)
                t1 = gp.tile([D, BC], BF16, tag="t1")
                nc.vector.tensor_tensor(t1, rz_bf[:, 0:BC], n_ps[:, 0:BC], OP.mult)
                t2 = gp.tile([D, BC], F32, tag="t2")
                nc.vector.tensor_tensor(t2, t1, n_ps[:, BC:2 * BC], OP.add)
                n_bf = gp.tile([D, BC], BF16, tag="nbf")
                nc.scalar.activation(n_bf, t2, AF.Tanh)
                dd = gp.tile([D, BC], BF16, tag="dd")
                nc.vector.tensor_sub(dd, hd_bf, n_bf)
                zd = gp.tile([D, BC], BF16, tag="zd")
                nc.vector.tensor_mul(zd, rz_bf[:, BC:2 * BC], dd)
                nc.vector.tensor_add(hd_bf, zd, n_bf)
                # heads
                mv1 = gq.tile([D, BC], F32, tag="mv1")
                MM(mv1, ws["wmv1T"], hd_bf, start=True, stop=not flags["mvb1"])
                if flags["mvb1"]:
                    MM(mv1, ws["mvb1"], ones_1b_f, start=False, stop=True)
                e1 = gp.tile([D, BC], BF16, tag="e1")
                nc.scalar.activation(e1, mv1, AF.Erf, scale=ISQ2)
                ge = gp.tile([D, BC], BF16, tag="ge")
                nc.vector.scalar_tensor_tensor(ge, e1, 1.0, mv1,
                                               op0=OP.add, op1=OP.mult)
                mv2 = gq1.tile([BC, 10], F32, tag="mv2")
                MM(mv2, ge, ws["wmv2"], start=True, stop=not flags["mvb2"])
                if flags["mvb2"]:
                    MM(mv2, ones_1b_bf, ws["mvb2"], start=False, stop=True)
                mu = gp.tile([BC, M], F32, tag="mu")
                nc.scalar.activation(mu, mv2[:, 0:M], AF.Tanh)
                vsb = gp.tile([BC, M], F32, tag="vsb")
                nc.vector.tensor_copy(vsb, mv2[:, M:2 * M])
                vv = gp.tile([BC, M], F32, tag="vv")
                nc.vector.tensor_mul(vv, vsb, vsb)
                p1 = gp.tile([BC, M], F32, tag="p1")
                nc.vector.tensor_scalar(p1, vsb, a1, a0,
                                        op0=OP.mult, op1=OP.add)
                sig = gp.tile([BC, M], F32, tag="sig")
                nc.vector.scalar_tensor_tensor(sig, vv, a2, p1,
                                               op0=OP.mult, op1=OP.add)
                pr_slot = preds_all[:, t * M:(t + 1) * M]
                nc.vector.tensor_mul(pr_slot, mu, sig)
                prT = gq1.tile([M, BC], F32, tag="prT")
                nc.tensor.transpose(prT, pr_slot, i128f)
                copy(pred_bf, prT, t)
        nc.sync.dma_start(d_out[:], preds_all)


# ======================================================================
# Self-contained driver: kernel(**inputs) -> np.ndarray [1024, 90, 5]
# ======================================================================
import sys as _sys
for _p in ("/opt/trn_rl_repo", "/root/.axon_site/_ro/trn_rl_repo"):
    if _p not in _sys.path:
        _sys.path.insert(0, _p)

_CACHE = {}


def _get_nc():
    if "nc" in _CACHE:
        return _CACHE["nc"], _CACHE["w_template"]
    return None, None


def kernel(**inputs):
    import concourse.bacc as bacc
    from concourse.bass_utils import run_bass_kernel_spmd

    w, cores = host_prep(inputs)
    nc = _CACHE.get("nc")
    if nc is None:
        nc = bacc.Bacc("TRN2", target_bir_lowering=False, debug=False,
                       num_devices=NCORES)
        build(nc, w)
        nc.compile()
        _CACHE["nc"] = nc
    in_maps = []
    for c in range(NCORES):
        m = {k: v for k, v in w.items() if isinstance(v, np.ndarray)}
        m.update(cores[c])
        in_maps.append(m)
    res = run_bass_kernel_spmd(nc, in_maps, core_ids=list(range(NCORES)))
    outs = [res.results[c]["preds"].reshape(BC, HOR, M) for c in range(NCORES)]
    return np.concatenate(outs, axis=0).astype(np.float32)

